# revision 10
# baseline (speedup 1.0000x reference)
"""Single-head causal attention (B=4, S=2048, M=H=1024) on 8 Trainium2 cores.

Sharding: core = (batch, half). Each core handles one batch and half its
queries. To balance the causal triangle, query 128-blocks are interleaved
stride-2: core half c owns global q-blocks {c, c+2, ..., c+14}, grouped in
4 chunks of 256 queries; chunk j = global blocks {4j+c, 4j+c+2} and attends
key blocks [0, 4j+4) — the last 4 get data-driven causal masks, so the one
compiled program serves both halves (SPMD).

Fast path (zero QK bias, v2, all-bf16): A = Wq^T Wk folded on host;
  q~ = A^T qT;  scoresT[k,q] = kT^T q~;  e = exp(scoresT/32)*mask;
  evT[m,q] = v^T e  (v in natural [S,M] layout — V never projected);
  out[q,h] = (evT^T WvT) / denom.
General path (nonzero QK bias) keeps the v1 design.
"""

import os

import numpy as np

B, S, MD, HD = 4, 2048, 1024, 1024
P = 128
NB = S // P            # 16 key/query blocks per batch
NCH = 4                # q-chunks of 256 per core
SQL = S // 2           # 1024 local queries per core
N_CORES = 8


def _build_general(use_pad: bool, use_vbias: bool):
    import concourse.bacc as bacc
    import concourse.mybir as mybir
    import concourse.tile as tile

    f32 = mybir.dt.float32
    f32r = mybir.dt.float32r
    bf16 = mybir.dt.bfloat16
    Act = mybir.ActivationFunctionType

    nc = bacc.Bacc("TRN2", num_swdge_queues=4, dynamic_dma_scratch_size=2048)

    qt = nc.dram_tensor("qt", [MD, SQL], f32r, kind="ExternalInput")
    kt = nc.dram_tensor("kt", [MD, S], f32r, kind="ExternalInput")
    vt = nc.dram_tensor("vt", [MD, S], f32r, kind="ExternalInput")
    wqt = nc.dram_tensor("wqt", [MD, HD], f32r, kind="ExternalInput")
    wkt = nc.dram_tensor("wkt", [MD, HD], f32r, kind="ExternalInput")
    wvt = nc.dram_tensor("wvt", [MD, HD], f32r, kind="ExternalInput")
    bq = nc.dram_tensor("bq", [HD], f32, kind="ExternalInput")
    bk = nc.dram_tensor("bk", [HD], f32, kind="ExternalInput")
    masks = nc.dram_tensor("masks", [4, P, 256], bf16, kind="ExternalInput")
    if use_pad:
        padm = nc.dram_tensor("padm", [P, NB], f32, kind="ExternalInput")
    if use_vbias:
        bv = nc.dram_tensor("bv", [HD], f32, kind="ExternalInput")
    out = nc.dram_tensor("out", [SQL, HD], f32, kind="ExternalOutput")

    MC = MD // P   # 8 contraction chunks
    HB = HD // P   # 8 h-blocks (partition dim of qhT/khT)

    with tile.TileContext(nc) as tc:
        with (
            tc.tile_pool(name="res", bufs=1) as res,
            tc.tile_pool(name="w", bufs=10) as wpool,
            tc.tile_pool(name="xin", bufs=4) as xin,
            tc.tile_pool(name="exp", bufs=16) as epool,
            tc.tile_pool(name="outp", bufs=1) as outp,
            tc.tile_pool(name="small", bufs=2) as small,
            tc.tile_pool(name="mm", bufs=5, space="PSUM") as mmp,
            tc.tile_pool(name="sc", bufs=2, space="PSUM") as scp,
            tc.tile_pool(name="dn", bufs=1, space="PSUM") as dnp,
        ):
            qh = res.tile([P, HB, SQL], f32r, tag="qh")
            kh = res.tile([P, HB, S], f32r, tag="kh")
            vh = res.tile([P, NB, HD], bf16, tag="vh")
            mt = res.tile([P, 4, 256], bf16, tag="mt")
            nc.scalar.dma_start(mt[:], masks.ap().rearrange("i p n -> p i n"))
            ones = res.tile([P, 2], bf16, tag="ones")
            nc.vector.memset(ones[:], 1.0)
            bias_t = res.tile([P, 2 * HB], f32, tag="bias")
            bqt = bias_t[:, 0:HB]
            nc.gpsimd.dma_start(bqt[:], bq.ap().rearrange("(hb p) -> p hb", p=P))
            bkt = bias_t[:, HB:2 * HB]
            nc.gpsimd.dma_start(bkt[:], bk.ap().rearrange("(hb p) -> p hb", p=P))
            if use_pad:
                pad_t = res.tile([P, NB], f32, tag="pad")
                nc.gpsimd.dma_start(pad_t[:], padm.ap())
            if use_vbias:
                ones_row = res.tile([1, P], f32r, tag="or")
                bvr = res.tile([1, HD], f32r, tag="bvr")
                nc.gpsimd.memset(ones_row[:].bitcast(f32), 1.0)
                nc.gpsimd.dma_start(bvr[:], bv.ap()[None, :])

            def load_w(dram, split=True):
                tiles = []
                for mc in range(MC):
                    t = wpool.tile([P, HD], f32r, tag="w", name=f"w{mc}")
                    weng = nc.scalar if (mc % 2 == 0 or not split) else nc.sync
                    weng.dma_start(t[:], dram.ap()[mc * P:(mc + 1) * P, :])
                    tiles.append(t)
                return tiles

            class XPair:
                def __init__(self, a, b):
                    self.a, self.b = a, b

                def __getitem__(self, key):
                    _, mc, cols = key
                    t = self.a if mc < 4 else self.b
                    return t[:, mc % 4, cols]

            def load_x(dram, c0):
                r = dram.ap().rearrange("(mc p) s -> p mc s", p=P)
                a = xin.tile([P, 4, 512], f32r, tag="x", name="xa")
                nc.sync.dma_start(a[:], r[:, 0:4, c0:c0 + 512])
                b = xin.tile([P, 4, 512], f32r, tag="x", name="xb")
                nc.sync.dma_start(b[:], r[:, 4:8, c0:c0 + 512])
                return XPair(a, b)

            # ---- Q projection: qh[:, hb, sq] (h on partitions) ----
            wq_t = load_w(wqt, split=False)
            for sqc in range(SQL // 512):
                xts = load_x(qt, sqc * 512)
                for hb in range(HB):
                    ps = mmp.tile([P, 512], f32, tag="mm")
                    for mc in range(MC):
                        nc.tensor.matmul(
                            ps[:], wq_t[mc][:, hb * P:(hb + 1) * P], xts[:, mc, :],
                            start=(mc == 0), stop=(mc == MC - 1))
                    nc.vector.tensor_scalar_add(
                        qh[:, hb, sqc * 512:(sqc + 1) * 512], ps[:],
                        bqt[:, hb:hb + 1])

            # ---- K projection: kh[:, hb, sk] ----
            wk_t = load_w(wkt)
            for skc in range(S // 512):
                xts = load_x(kt, skc * 512)
                for hb in range(HB):
                    ps = mmp.tile([P, 512], f32, tag="mm")
                    for mc in range(MC):
                        nc.tensor.matmul(
                            ps[:], wk_t[mc][:, hb * P:(hb + 1) * P], xts[:, mc, :],
                            start=(mc == 0), stop=(mc == MC - 1))
                    nc.vector.tensor_scalar_add(
                        kh[:, hb, skc * 512:(skc + 1) * 512], ps[:],
                        bkt[:, hb:hb + 1])

            # ---- V projection: vh[:, skb, h] (keys on partitions) ----
            wv_t = load_w(wvt)
            for skc in range(S // 512):
                xts = load_x(vt, skc * 512)
                for sbl in range(4):
                    skb = skc * 4 + sbl
                    for hc in range(2):
                        ps = mmp.tile([P, 512], f32, tag="mm")
                        for mc in range(MC):
                            nc.tensor.matmul(
                                ps[:], xts[:, mc, sbl * P:(sbl + 1) * P],
                                wv_t[mc][:, hc * 512:(hc + 1) * 512],
                                start=(mc == 0),
                                stop=(mc == MC - 1) and not use_vbias)
                        if use_vbias:
                            nc.tensor.matmul(
                                ps[:], ones_row[:],
                                bvr[:, hc * 512:(hc + 1) * 512],
                                start=False, stop=True)
                        nc.vector.tensor_copy(vh[:, skb, hc * 512:(hc + 1) * 512], ps[:])


            # ---- attention, chunk j = 256 queries, keys [0, (4j+4)*128) ----
            for j in range(NCH):
                E = 4 * j + 4
                sq0 = j * 256
                exps = []
                for kb in range(E):
                    sps = scp.tile([P, 256], f32, tag="s")
                    for hb in range(HB):
                        nc.tensor.matmul(
                            sps[:], kh[:, hb, kb * P:(kb + 1) * P],
                            qh[:, hb, sq0:sq0 + 256],
                            start=(hb == 0), stop=(hb == HB - 1))
                    ex = epool.tile([P, 256], bf16, tag="e")
                    nc.scalar.activation(ex[:], sps[:], Act.Exp, scale=1.0 / 32.0)
                    if kb >= 4 * j:
                        nc.vector.tensor_mul(ex[:], ex[:], mt[:, kb - 4 * j, :])
                    if use_pad:
                        nc.vector.tensor_scalar_mul(ex[:], ex[:], pad_t[:, kb:kb + 1])
                    exps.append(ex)

                for t in range(2):
                    dps = dnp.tile([P, 2], f32, tag="d")
                    avs = [mmp.tile([P, 512], f32, tag="mm", name=f"av{j}_{t}_{hc2}")
                           for hc2 in range(2)]
                    for kb in range(E):
                        lhs = exps[kb][:, t * P:(t + 1) * P]
                        for hc in range(2):
                            nc.tensor.matmul(
                                avs[hc][:], lhs, vh[:, kb, hc * 512:(hc + 1) * 512],
                                start=(kb == 0), stop=(kb == E - 1))
                        nc.tensor.matmul(
                            dps[:], lhs, ones[:],
                            start=(kb == 0), stop=(kb == E - 1))
                    dr = small.tile([P, 2], f32, tag="dr")
                    nc.vector.tensor_copy(dr[:, 0:1], dps[:, 0:1])
                    rr = dr[:, 1:2]
                    nc.vector.reciprocal(rr[:], dr[:, 0:1])
                    o = outp.tile([P, HD], f32, tag="o")
                    for hc in range(2):
                        nc.vector.tensor_scalar_mul(
                            o[:, hc * 512:(hc + 1) * 512], avs[hc][:], rr[:])
                    lb = 2 * j + t
                    nc.sync.dma_start(out.ap()[lb * P:(lb + 1) * P, :], o[:])

    nc.compile()
    return nc



def _build_folded(use_pad: bool, use_vbias: bool):
    """Zero-QK-bias fast path, v2 (all-bf16, no V projection):
      q~ = A^T qT (A = Wq^T Wk folded on host)
      scoresT[k, q] = kT^T @ q~          (contraction over m)
      e = exp(scoresT/32) * causal_mask  (bf16)
      evT[m, q] = v^T @ e                (contraction over keys; v in natural
                                          [S, M] layout so no transpose anywhere)
      out[q, h] = (evT^T @ WvT) / denom  (contraction over m)
    The V projection (v @ WvT over ALL keys) never happens: applying WvT after
    the attention-weighted sum only touches the causal 5/8 of the key mass and
    runs over queries (no inter-core duplication)."""
    import concourse.bacc as bacc
    import concourse.mybir as mybir
    import concourse.tile as tile

    f32 = mybir.dt.float32
    bf16 = mybir.dt.bfloat16
    Act = mybir.ActivationFunctionType

    nc = bacc.Bacc("TRN2", num_swdge_queues=4, dynamic_dma_scratch_size=2048)

    # host-packed so every DMA is >=2KB-contiguous on both sides:
    # at[hq,p,mc,c] = A[mc*128+p, hq*256+c];  qt[sqc,p,mc,c] = qT[mc*128+p, sqc*512+c]
    # kt[kq,p,mc,c] = kT[mc*128+p, kq*256+c]
    qt = nc.dram_tensor("qt", [2, P, MD // P, 512], bf16, kind="ExternalInput")
    at = nc.dram_tensor("at", [4, P, MD // P, 256], bf16, kind="ExternalInput")
    kt = nc.dram_tensor("kt", [8, P, MD // P, 256], bf16, kind="ExternalInput")
    vn = nc.dram_tensor("vn", [S, MD], bf16, kind="ExternalInput")
    wvt = nc.dram_tensor("wvt", [MD, HD], bf16, kind="ExternalInput")
    masks = nc.dram_tensor("masks", [4, P, 256], bf16, kind="ExternalInput")
    if use_pad:
        padm = nc.dram_tensor("padm", [P, NB], f32, kind="ExternalInput")
    if use_vbias:
        bv = nc.dram_tensor("bv", [HD], f32, kind="ExternalInput")
    out = nc.dram_tensor("out", [SQL, HD], f32, kind="ExternalOutput")

    MC = MD // P   # 8 contraction chunks over m

    with tile.TileContext(nc) as tc:
        with (
            tc.tile_pool(name="res", bufs=1) as res,
            tc.tile_pool(name="exp", bufs=30) as epool,
            tc.tile_pool(name="evs", bufs=2) as evs,
            tc.tile_pool(name="outp", bufs=2) as outp,
            tc.tile_pool(name="small", bufs=4) as small,
        ):
            a_sb = res.tile([P, 4, MC, 256], bf16, tag="a")
            qx = res.tile([P, 2, MC, 512], bf16, tag="qx")
            qh = res.tile([P, MC, SQL], bf16, tag="qh")
            k_sb = res.tile([P, 8, MC, 256], bf16, tag="k")
            v_sb = res.tile([P, NB, MD], bf16, tag="v")
            wv_sb = res.tile([P, MC, HD], bf16, tag="wv")
            mt = res.tile([P, 4, 256], bf16, tag="mt")
            ones = res.tile([P, 2], bf16, tag="ones")
            wmt = res.tile([P, P], bf16, tag="wm")
            nc.vector.memset(ones[:], 1.0)
            nc.vector.memset(wmt[:], 0.0)

            def ablk(hb, mc):
                return a_sb[:, hb // 2, mc, (hb % 2) * P:(hb % 2 + 1) * P]

            def kblk(kb, mc):
                return k_sb[:, kb // 2, mc, (kb % 2) * P:(kb % 2 + 1) * P]

            # input DMAs, priority order. sync: A then k then outs;
            # scalar: q then wv; gpsimd: v (kb-ascending) then masks.
            for hq in range(4):
                nc.sync.dma_start(a_sb[:, hq, :, :], at.ap()[hq])
            for sqc in range(2):
                nc.scalar.dma_start(qx[:, sqc, :, :], qt.ap()[sqc])
            for kq in range(8):
                nc.sync.dma_start(k_sb[:, kq, :, :], kt.ap()[kq])
            vr = vn.ap().rearrange("(kb p) m -> p kb m", p=P)
            for vq in range(8):
                nc.gpsimd.dma_start(v_sb[:, 2 * vq:2 * vq + 2, :],
                                    vr[:, 2 * vq:2 * vq + 2, :])
            wr = wvt.ap().rearrange("(mc p) h -> p mc h", p=P)
            for wc in range(2):
                nc.scalar.dma_start(wv_sb[:, 4 * wc:4 * wc + 4, :],
                                    wr[:, 4 * wc:4 * wc + 4, :])
            nc.gpsimd.dma_start(mt[:], masks.ap().rearrange("i p n -> p i n"))
            if use_pad:
                pad_t = res.tile([P, NB], f32, tag="pad")
                nc.gpsimd.dma_start(pad_t[:], padm.ap())
            if use_vbias:
                bvr = res.tile([1, HD], f32, tag="bvr")
                nc.gpsimd.dma_start(bvr[:], bv.ap()[None, :])
                ones_row = res.tile([1, P], bf16, tag="or")
                nc.gpsimd.memset(ones_row[:], 1.0)
                bvb = res.tile([P, HD], f32, tag="bvb")

            # ---- Q-rot: q~[m', sq] = A^T @ qT, bf16 out ----
            with tc.tile_pool(name="qr", bufs=4, space="PSUM") as qrp:
                # HAM warmup: ~3.4us of throwaway matmuls while input DMAs
                # stream, so real matmuls start at the full 2.4 GHz clock.
                wps = qrp.tile([P, P], f32, tag="q", name="wps")
                for w in range(32):
                    nc.tensor.matmul(wps[:], wmt[:], wmt[:],
                                     start=(w == 0), stop=(w == 31))
                for sqc in range(SQL // 512):
                    for hb in range(MC):
                        ps = qrp.tile([P, 512], f32, tag="q",
                                      name=f"qp{sqc}_{hb}")
                        for mc in range(MC):
                            nc.tensor.matmul(
                                ps[:], ablk(hb, mc),
                                qx[:, sqc, mc, :],
                                start=(mc == 0), stop=(mc == MC - 1))
                        if hb % 2 == 0:
                            nc.vector.tensor_copy(
                                qh[:, hb, sqc * 512:(sqc + 1) * 512], ps[:])
                        else:
                            nc.scalar.copy(
                                qh[:, hb, sqc * 512:(sqc + 1) * 512], ps[:])
                if use_vbias:
                    # broadcast bv across partitions once: ones_row^T @ bvr
                    for hc in range(2):
                        bps = qrp.tile([P, 512], f32, tag="q", name=f"bb{hc}")
                        nc.tensor.matmul(bps[:], ones_row[:],
                                         bvr[:, hc * 512:(hc + 1) * 512],
                                         start=True, stop=True)
                        nc.vector.tensor_copy(
                            bvb[:, hc * 512:(hc + 1) * 512], bps[:])

            # ---- attention, chunk j = 256 queries, keys [0, (4j+4)*128) ----
            with (
                tc.tile_pool(name="sc", bufs=2, space="PSUM") as scp,
                tc.tile_pool(name="ev", bufs=4, space="PSUM") as evp,
                tc.tile_pool(name="dn", bufs=1, space="PSUM") as dnp,
                tc.tile_pool(name="fin", bufs=1, space="PSUM") as finp,
            ):
                pending = []

                def make_finals(j, exps, evt):
                    """Deferred denominator + final-GEMM thunks for chunk j
                    (issued interleaved into chunk j+1's score loop so the
                    single fin PSUM bank never stalls the PE)."""
                    E = 4 * j + 4
                    st = {}

                    def dn_chain(t):
                        def f():
                            dps = dnp.tile([P, 2], f32, tag="d",
                                           name=f"d{j}_{t}")
                            for kb in range(E):
                                nc.tensor.matmul(
                                    dps[:], exps[kb][:, t * P:(t + 1) * P],
                                    ones[:], start=(kb == 0),
                                    stop=(kb == E - 1))
                            dr = small.tile([P, 2], f32, tag="dr",
                                            name=f"dr{j}_{t}")
                            nc.vector.reciprocal(dr[:, 1:2], dps[:, 0:1])
                            st[t] = dr[:, 1:2]
                            st[(t, "o")] = outp.tile([P, HD], f32, tag="o",
                                                     name=f"o{j}_{t}")
                        return f

                    def fin_group(t, hc):
                        def f():
                            fps = finp.tile([P, 512], f32, tag="f",
                                            name=f"f{j}_{t}_{hc}")
                            for mc in range(MC):
                                nc.tensor.matmul(
                                    fps[:], evt[:, mc, t * P:(t + 1) * P],
                                    wv_sb[:, mc, hc * 512:(hc + 1) * 512],
                                    start=(mc == 0), stop=(mc == MC - 1))
                            o = st[(t, "o")]
                            rr = st[t]
                            osl = o[:, hc * 512:(hc + 1) * 512]
                            if hc == 0:
                                nc.vector.tensor_scalar_mul(osl, fps[:], rr)
                            else:
                                nc.scalar.activation(osl, fps[:], Act.Copy,
                                                     scale=rr)
                            if use_vbias:
                                nc.vector.tensor_add(
                                    osl, osl, bvb[:, hc * 512:(hc + 1) * 512])
                            if hc == 1:
                                lb = 2 * j + t
                                nc.sync.dma_start(
                                    out.ap()[lb * P:(lb + 1) * P, :], o[:])
                        return f

                    return [dn_chain(0), fin_group(0, 0), dn_chain(1),
                            fin_group(0, 1), fin_group(1, 0), fin_group(1, 1)]

                for j in range(NCH):
                    E = 4 * j + 4
                    sq0 = j * 256
                    exps = []
                    evt = evs.tile([P, MC, 256], bf16, tag="evt",
                                   name=f"evt{j}")
                    p0 = [evp.tile([P, 256], f32, tag="ev",
                                   name=f"ev{j}_0_{i}") for i in range(2)]
                    for kb in range(E):
                        sps = scp.tile([P, 256], f32, tag="s",
                                       name=f"s{j}_{kb}")
                        for mc in range(MC):
                            nc.tensor.matmul(
                                sps[:], kblk(kb, mc),
                                qh[:, mc, sq0:sq0 + 256],
                                start=(mc == 0), stop=(mc == MC - 1))
                        ex = epool.tile([P, 256], bf16, tag="e",
                                        name=f"e{j}_{kb}")
                        nc.scalar.activation(ex[:], sps[:], Act.Exp,
                                             scale=1.0 / 32.0)
                        if kb >= 4 * j:
                            nc.vector.tensor_mul(ex[:], ex[:],
                                                 mt[:, kb - 4 * j, :])
                        if use_pad:
                            nc.vector.tensor_scalar_mul(
                                ex[:], ex[:], pad_t[:, kb:kb + 1])
                        exps.append(ex)
                        # evT pass 0 (m-chunks 0,1) rides the score loop one
                        # kb behind so the PE never waits on the just-issued
                        # exp; previous chunk's finals fill the other slots.
                        if kb > 0:
                            for i in range(2):
                                nc.tensor.matmul(
                                    p0[i][:], v_sb[:, kb - 1, i * P:(i + 1) * P],
                                    exps[kb - 1][:], start=(kb == 1),
                                    stop=False)
                        if pending:
                            pending.pop(0)()
                    for i in range(2):
                        nc.tensor.matmul(
                            p0[i][:], v_sb[:, E - 1, i * P:(i + 1) * P],
                            exps[E - 1][:], start=(E == 1), stop=True)
                        eng = nc.vector if i == 0 else nc.scalar
                        (eng.tensor_copy if i == 0 else eng.copy)(
                            evt[:, i, :], p0[i][:])
                    for pp in range(1, 4):
                        pts = [evp.tile([P, 256], f32, tag="ev",
                                        name=f"ev{j}_{pp}_{i}")
                               for i in range(2)]
                        for kb in range(E):
                            for i in range(2):
                                mc = pp * 2 + i
                                nc.tensor.matmul(
                                    pts[i][:], v_sb[:, kb, mc * P:(mc + 1) * P],
                                    exps[kb][:], start=(kb == 0),
                                    stop=(kb == E - 1))
                        for i in range(2):
                            mc = pp * 2 + i
                            if i == 0:
                                nc.vector.tensor_copy(evt[:, mc, :], pts[i][:])
                            else:
                                nc.scalar.copy(evt[:, mc, :], pts[i][:])
                    while pending:
                        pending.pop(0)()
                    pending = make_finals(j, exps, evt)
                while pending:
                    pending.pop(0)()

    nc.compile()
    return nc

def kernel(q, k, v, attention_mask, Wq_w, Wq_b, Wk_w, Wk_b, Wv_w, Wv_b):
    import ml_dtypes
    from concourse.bass_utils import run_bass_kernel_spmd

    q = np.asarray(q, dtype=np.float32)
    k = np.asarray(k, dtype=np.float32)
    v = np.asarray(v, dtype=np.float32)
    am = np.asarray(attention_mask)

    use_pad = not bool((am == 1).all())
    use_vbias = bool(np.any(np.asarray(Wv_b) != 0))

    use_qkbias = bool(np.any(np.asarray(Wq_b) != 0) or np.any(np.asarray(Wk_b) != 0))
    if use_qkbias:
        nc = _build_general(use_pad, use_vbias)
    else:
        nc = _build_folded(use_pad, use_vbias)

    bf16 = ml_dtypes.bfloat16
    wvt = np.ascontiguousarray(np.asarray(Wv_w, np.float32).T)
    if not use_qkbias:
        A = (np.asarray(Wq_w, np.float64).T @ np.asarray(Wk_w, np.float64))
        A = np.ascontiguousarray(A.astype(np.float32))
    bq = np.ascontiguousarray(np.asarray(Wq_b, np.float32))
    bk = np.ascontiguousarray(np.asarray(Wk_b, np.float32))
    bv = np.ascontiguousarray(np.asarray(Wv_b, np.float32))

    # causal masks for the 4 tail key-blocks of each chunk, per half c.
    # entry [i, a, col]: key (4j+i)*128+a vs query (4j+c+2t)*128+b, t=col//128.
    mask_c = []
    a = np.arange(P)[:, None]
    col = np.arange(256)[None, :]
    for c in range(2):
        t = col // P
        b_ = col % P
        m = np.stack([
            (128 * i + a <= 128 * (c + 2 * t) + b_) for i in range(4)
        ]).astype(np.float32)
        mask_c.append(m.astype(bf16))

    perms = []
    for c in range(2):
        perm = np.concatenate([
            np.arange(P) + (4 * j + c + 2 * t) * P
            for j in range(NCH) for t in range(2)
        ])
        perms.append(perm)

    in_maps = []
    if use_qkbias:
        wqt = np.ascontiguousarray(np.asarray(Wq_w, np.float32).T)
        wkt = np.ascontiguousarray(np.asarray(Wk_w, np.float32).T)
        kT = [np.ascontiguousarray(k[b].T) for b in range(B)]
        vT = [np.ascontiguousarray(v[b].T) for b in range(B)]
        for cid in range(N_CORES):
            b, c = cid // 2, cid % 2
            qT = np.ascontiguousarray(q[b].T[:, perms[c]])
            m = dict(qt=qT, kt=kT[b], vt=vT[b], wqt=wqt, wkt=wkt, wvt=wvt,
                     bq=bq, bk=bk, masks=mask_c[c])
            if use_pad:
                padv = am[b].astype(np.float32)
                m["padm"] = np.ascontiguousarray(padv.reshape(NB, P).T)
            if use_vbias:
                m["bv"] = bv
            in_maps.append(m)
    else:
        # pack so each device DMA is contiguous: x[q, p, mc, c] = xT[mc*128+p, q*W+c]
        def pack(xT, W):
            m2 = xT.shape[1] // W
            return np.ascontiguousarray(
                xT.reshape(8, 128, m2, W).transpose(2, 1, 0, 3))

        Ab = pack(A.astype(bf16), 256)
        wvtb = wvt.astype(bf16)
        kTb = [pack(np.ascontiguousarray(k[b].T).astype(bf16), 256)
               for b in range(B)]
        vnb = [np.ascontiguousarray(v[b]).astype(bf16) for b in range(B)]
        for cid in range(N_CORES):
            b, c = cid // 2, cid % 2
            qTb = pack(np.ascontiguousarray(q[b].T[:, perms[c]]).astype(bf16),
                       512)
            m = dict(qt=qTb, at=Ab, kt=kTb[b], vn=vnb[b], wvt=wvtb,
                     masks=mask_c[c])
            if use_pad:
                padv = am[b].astype(np.float32)
                m["padm"] = np.ascontiguousarray(padv.reshape(NB, P).T)
            if use_vbias:
                m["bv"] = bv
            in_maps.append(m)

    prof_dir = os.environ.get("ATTN_PROF_DIR")
    if prof_dir:
        try:
            from antenv.axon_hooks import get_axon_ntff_profile_hook
            hook = get_axon_ntff_profile_hook()
        except ImportError:
            hook = None
        if hook is None:
            from trn_agent_boot.trn_boot import _ntff_profile_via_ctypes
            hook = _ntff_profile_via_ctypes("/opt/axon/libaxon_pjrt.so")
        with hook(prof_dir, [0]):
            res = run_bass_kernel_spmd(nc, in_maps, list(range(N_CORES)))
    else:
        res = run_bass_kernel_spmd(nc, in_maps, list(range(N_CORES)))

    out = np.empty((B, S, HD), np.float32)
    for cid in range(N_CORES):
        b, c = cid // 2, cid % 2
        oc = res.results[cid]["out"]
        out[b, perms[c], :] = oc
    return out



# revision 16
# speedup vs baseline: 1.1754x; 1.1754x over previous
"""Single-head causal attention (B=4, S=2048, M=H=1024) on 8 Trainium2 cores.

Sharding: core = (batch, half). Each core handles one batch and half its
queries. To balance the causal triangle, query 128-blocks are interleaved
stride-2: core half c owns global q-blocks {c, c+2, ..., c+14}, grouped in
4 chunks of 256 queries; chunk j = global blocks {4j+c, 4j+c+2} and attends
key blocks [0, 4j+4) — the last 4 get data-driven causal masks, so the one
compiled program serves both halves (SPMD).

Fast path (zero QK bias, v2, all-bf16): A = Wq^T Wk folded on host;
  q~ = A^T qT;  scoresT[k,q] = kT^T q~;  e = exp(scoresT/32)*mask;
  evT[m,q] = v^T e  (v in natural [S,M] layout — V never projected);
  out[q,h] = (evT^T WvT) / denom.
General path (nonzero QK bias) keeps the v1 design.
"""

import os

import numpy as np

B, S, MD, HD = 4, 2048, 1024, 1024
P = 128
NB = S // P            # 16 key/query blocks per batch
NCH = 4                # q-chunks of 256 per core
SQL = S // 2           # 1024 local queries per core
N_CORES = 8


def _build_general(use_pad: bool, use_vbias: bool):
    import concourse.bacc as bacc
    import concourse.mybir as mybir
    import concourse.tile as tile

    f32 = mybir.dt.float32
    f32r = mybir.dt.float32r
    bf16 = mybir.dt.bfloat16
    Act = mybir.ActivationFunctionType

    nc = bacc.Bacc("TRN2", num_swdge_queues=4, dynamic_dma_scratch_size=2048)

    qt = nc.dram_tensor("qt", [MD, SQL], f32r, kind="ExternalInput")
    kt = nc.dram_tensor("kt", [MD, S], f32r, kind="ExternalInput")
    vt = nc.dram_tensor("vt", [MD, S], f32r, kind="ExternalInput")
    wqt = nc.dram_tensor("wqt", [MD, HD], f32r, kind="ExternalInput")
    wkt = nc.dram_tensor("wkt", [MD, HD], f32r, kind="ExternalInput")
    wvt = nc.dram_tensor("wvt", [MD, HD], f32r, kind="ExternalInput")
    bq = nc.dram_tensor("bq", [HD], f32, kind="ExternalInput")
    bk = nc.dram_tensor("bk", [HD], f32, kind="ExternalInput")
    masks = nc.dram_tensor("masks", [4, P, 256], bf16, kind="ExternalInput")
    if use_pad:
        padm = nc.dram_tensor("padm", [P, NB], f32, kind="ExternalInput")
    if use_vbias:
        bv = nc.dram_tensor("bv", [HD], f32, kind="ExternalInput")
    out = nc.dram_tensor("out", [SQL, HD], f32, kind="ExternalOutput")

    MC = MD // P   # 8 contraction chunks
    HB = HD // P   # 8 h-blocks (partition dim of qhT/khT)

    with tile.TileContext(nc) as tc:
        with (
            tc.tile_pool(name="res", bufs=1) as res,
            tc.tile_pool(name="w", bufs=10) as wpool,
            tc.tile_pool(name="xin", bufs=4) as xin,
            tc.tile_pool(name="exp", bufs=16) as epool,
            tc.tile_pool(name="outp", bufs=1) as outp,
            tc.tile_pool(name="small", bufs=2) as small,
            tc.tile_pool(name="mm", bufs=5, space="PSUM") as mmp,
            tc.tile_pool(name="sc", bufs=2, space="PSUM") as scp,
            tc.tile_pool(name="dn", bufs=1, space="PSUM") as dnp,
        ):
            qh = res.tile([P, HB, SQL], f32r, tag="qh")
            kh = res.tile([P, HB, S], f32r, tag="kh")
            vh = res.tile([P, NB, HD], bf16, tag="vh")
            mt = res.tile([P, 4, 256], bf16, tag="mt")
            nc.scalar.dma_start(mt[:], masks.ap().rearrange("i p n -> p i n"))
            ones = res.tile([P, 2], bf16, tag="ones")
            nc.vector.memset(ones[:], 1.0)
            bias_t = res.tile([P, 2 * HB], f32, tag="bias")
            bqt = bias_t[:, 0:HB]
            nc.gpsimd.dma_start(bqt[:], bq.ap().rearrange("(hb p) -> p hb", p=P))
            bkt = bias_t[:, HB:2 * HB]
            nc.gpsimd.dma_start(bkt[:], bk.ap().rearrange("(hb p) -> p hb", p=P))
            if use_pad:
                pad_t = res.tile([P, NB], f32, tag="pad")
                nc.gpsimd.dma_start(pad_t[:], padm.ap())
            if use_vbias:
                ones_row = res.tile([1, P], f32r, tag="or")
                bvr = res.tile([1, HD], f32r, tag="bvr")
                nc.gpsimd.memset(ones_row[:].bitcast(f32), 1.0)
                nc.gpsimd.dma_start(bvr[:], bv.ap()[None, :])

            def load_w(dram, split=True):
                tiles = []
                for mc in range(MC):
                    t = wpool.tile([P, HD], f32r, tag="w", name=f"w{mc}")
                    weng = nc.scalar if (mc % 2 == 0 or not split) else nc.sync
                    weng.dma_start(t[:], dram.ap()[mc * P:(mc + 1) * P, :])
                    tiles.append(t)
                return tiles

            class XPair:
                def __init__(self, a, b):
                    self.a, self.b = a, b

                def __getitem__(self, key):
                    _, mc, cols = key
                    t = self.a if mc < 4 else self.b
                    return t[:, mc % 4, cols]

            def load_x(dram, c0):
                r = dram.ap().rearrange("(mc p) s -> p mc s", p=P)
                a = xin.tile([P, 4, 512], f32r, tag="x", name="xa")
                nc.sync.dma_start(a[:], r[:, 0:4, c0:c0 + 512])
                b = xin.tile([P, 4, 512], f32r, tag="x", name="xb")
                nc.sync.dma_start(b[:], r[:, 4:8, c0:c0 + 512])
                return XPair(a, b)

            # ---- Q projection: qh[:, hb, sq] (h on partitions) ----
            wq_t = load_w(wqt, split=False)
            for sqc in range(SQL // 512):
                xts = load_x(qt, sqc * 512)
                for hb in range(HB):
                    ps = mmp.tile([P, 512], f32, tag="mm")
                    for mc in range(MC):
                        nc.tensor.matmul(
                            ps[:], wq_t[mc][:, hb * P:(hb + 1) * P], xts[:, mc, :],
                            start=(mc == 0), stop=(mc == MC - 1))
                    nc.vector.tensor_scalar_add(
                        qh[:, hb, sqc * 512:(sqc + 1) * 512], ps[:],
                        bqt[:, hb:hb + 1])

            # ---- K projection: kh[:, hb, sk] ----
            wk_t = load_w(wkt)
            for skc in range(S // 512):
                xts = load_x(kt, skc * 512)
                for hb in range(HB):
                    ps = mmp.tile([P, 512], f32, tag="mm")
                    for mc in range(MC):
                        nc.tensor.matmul(
                            ps[:], wk_t[mc][:, hb * P:(hb + 1) * P], xts[:, mc, :],
                            start=(mc == 0), stop=(mc == MC - 1))
                    nc.vector.tensor_scalar_add(
                        kh[:, hb, skc * 512:(skc + 1) * 512], ps[:],
                        bkt[:, hb:hb + 1])

            # ---- V projection: vh[:, skb, h] (keys on partitions) ----
            wv_t = load_w(wvt)
            for skc in range(S // 512):
                xts = load_x(vt, skc * 512)
                for sbl in range(4):
                    skb = skc * 4 + sbl
                    for hc in range(2):
                        ps = mmp.tile([P, 512], f32, tag="mm")
                        for mc in range(MC):
                            nc.tensor.matmul(
                                ps[:], xts[:, mc, sbl * P:(sbl + 1) * P],
                                wv_t[mc][:, hc * 512:(hc + 1) * 512],
                                start=(mc == 0),
                                stop=(mc == MC - 1) and not use_vbias)
                        if use_vbias:
                            nc.tensor.matmul(
                                ps[:], ones_row[:],
                                bvr[:, hc * 512:(hc + 1) * 512],
                                start=False, stop=True)
                        nc.vector.tensor_copy(vh[:, skb, hc * 512:(hc + 1) * 512], ps[:])


            # ---- attention, chunk j = 256 queries, keys [0, (4j+4)*128) ----
            for j in range(NCH):
                E = 4 * j + 4
                sq0 = j * 256
                exps = []
                for kb in range(E):
                    sps = scp.tile([P, 256], f32, tag="s")
                    for hb in range(HB):
                        nc.tensor.matmul(
                            sps[:], kh[:, hb, kb * P:(kb + 1) * P],
                            qh[:, hb, sq0:sq0 + 256],
                            start=(hb == 0), stop=(hb == HB - 1))
                    ex = epool.tile([P, 256], bf16, tag="e")
                    nc.scalar.activation(ex[:], sps[:], Act.Exp, scale=1.0 / 32.0)
                    if kb >= 4 * j:
                        nc.vector.tensor_mul(ex[:], ex[:], mt[:, kb - 4 * j, :])
                    if use_pad:
                        nc.vector.tensor_scalar_mul(ex[:], ex[:], pad_t[:, kb:kb + 1])
                    exps.append(ex)

                for t in range(2):
                    dps = dnp.tile([P, 2], f32, tag="d")
                    avs = [mmp.tile([P, 512], f32, tag="mm", name=f"av{j}_{t}_{hc2}")
                           for hc2 in range(2)]
                    for kb in range(E):
                        lhs = exps[kb][:, t * P:(t + 1) * P]
                        for hc in range(2):
                            nc.tensor.matmul(
                                avs[hc][:], lhs, vh[:, kb, hc * 512:(hc + 1) * 512],
                                start=(kb == 0), stop=(kb == E - 1))
                        nc.tensor.matmul(
                            dps[:], lhs, ones[:],
                            start=(kb == 0), stop=(kb == E - 1))
                    dr = small.tile([P, 2], f32, tag="dr")
                    nc.vector.tensor_copy(dr[:, 0:1], dps[:, 0:1])
                    rr = dr[:, 1:2]
                    nc.vector.reciprocal(rr[:], dr[:, 0:1])
                    o = outp.tile([P, HD], f32, tag="o")
                    for hc in range(2):
                        nc.vector.tensor_scalar_mul(
                            o[:, hc * 512:(hc + 1) * 512], avs[hc][:], rr[:])
                    lb = 2 * j + t
                    nc.sync.dma_start(out.ap()[lb * P:(lb + 1) * P, :], o[:])

    nc.compile()
    return nc



def _build_folded(use_pad: bool, use_vbias: bool):
    """Zero-QK-bias fast path, v2 (all-bf16, no V projection):
      q~ = A^T qT (A = Wq^T Wk folded on host)
      scoresT[k, q] = kT^T @ q~          (contraction over m)
      e = exp(scoresT/32) * causal_mask  (bf16)
      evT[m, q] = v^T @ e                (contraction over keys; v in natural
                                          [S, M] layout so no transpose anywhere)
      out[q, h] = (evT^T @ WvT) / denom  (contraction over m)
    The V projection (v @ WvT over ALL keys) never happens: applying WvT after
    the attention-weighted sum only touches the causal 5/8 of the key mass and
    runs over queries (no inter-core duplication)."""
    import concourse.bacc as bacc
    import concourse.mybir as mybir
    import concourse.tile as tile

    f32 = mybir.dt.float32
    bf16 = mybir.dt.bfloat16
    Act = mybir.ActivationFunctionType

    nc = bacc.Bacc("TRN2", num_swdge_queues=4, dynamic_dma_scratch_size=2048)

    # host-packed so every DMA is >=2KB-contiguous on both sides:
    # at[hq,p,mc,c] = A[mc*128+p, hq*256+c];  qt[sqc,p,mc,c] = qT[mc*128+p, sqc*512+c]
    # kt[kq,p,mc,c] = kT[mc*128+p, kq*256+c]
    qt = nc.dram_tensor("qt", [2, P, MD // P, 512], bf16, kind="ExternalInput")
    at = nc.dram_tensor("at", [4, P, MD // P, 256], bf16, kind="ExternalInput")
    kt = nc.dram_tensor("kt", [8, P, MD // P, 256], bf16, kind="ExternalInput")
    vn = nc.dram_tensor("vn", [S, MD], bf16, kind="ExternalInput")
    wvt = nc.dram_tensor("wvt", [MD, HD], bf16, kind="ExternalInput")
    masks = nc.dram_tensor("masks", [4, P, 256], bf16, kind="ExternalInput")
    if use_pad:
        padm = nc.dram_tensor("padm", [P, NB], f32, kind="ExternalInput")
    if use_vbias:
        bv = nc.dram_tensor("bv", [HD], f32, kind="ExternalInput")
    out = nc.dram_tensor("out", [SQL, HD], f32, kind="ExternalOutput")

    MC = MD // P   # 8 contraction chunks over m

    with tile.TileContext(nc) as tc:
        with (
            tc.tile_pool(name="res", bufs=1) as res,
            tc.tile_pool(name="exp", bufs=30) as epool,
            tc.tile_pool(name="evs", bufs=2) as evs,
            tc.tile_pool(name="outp", bufs=2) as outp,
            tc.tile_pool(name="small", bufs=4) as small,
        ):
            a_sb = res.tile([P, 4, MC, 256], bf16, tag="a")
            qx = res.tile([P, 2, MC, 512], bf16, tag="qx")
            qh = res.tile([P, MC, SQL], bf16, tag="qh")
            k_sb = res.tile([P, 8, MC, 256], bf16, tag="k")
            v_sb = res.tile([P, NB, MD], bf16, tag="v")
            wv_sb = res.tile([P, MC, HD], bf16, tag="wv")
            mt = res.tile([P, 4, 256], bf16, tag="mt")
            ones = res.tile([P, 2], bf16, tag="ones")
            wmt = res.tile([P, P], bf16, tag="wm")
            nc.vector.memset(ones[:], 1.0)
            nc.vector.memset(wmt[:], 0.0)

            def ablk(hb, mc):
                return a_sb[:, hb // 2, mc, (hb % 2) * P:(hb % 2 + 1) * P]

            def kblk(kb, mc):
                return k_sb[:, kb // 2, mc, (kb % 2) * P:(kb % 2 + 1) * P]

            # input DMAs, priority order. sync: A then k then outs;
            # scalar: q then wv; gpsimd: v (kb-ascending) then masks.
            for hq in range(4):
                nc.sync.dma_start(a_sb[:, hq, :, :], at.ap()[hq])
            for sqc in range(2):
                nc.scalar.dma_start(qx[:, sqc, :, :], qt.ap()[sqc])
            for kq in range(8):
                nc.sync.dma_start(k_sb[:, kq, :, :], kt.ap()[kq])
            vr = vn.ap().rearrange("(kb p) m -> p kb m", p=P)
            for vq in range(8):
                nc.gpsimd.dma_start(v_sb[:, 2 * vq:2 * vq + 2, :],
                                    vr[:, 2 * vq:2 * vq + 2, :])
            wr = wvt.ap().rearrange("(mc p) h -> p mc h", p=P)
            for wc in range(2):
                nc.scalar.dma_start(wv_sb[:, 4 * wc:4 * wc + 4, :],
                                    wr[:, 4 * wc:4 * wc + 4, :])
            nc.gpsimd.dma_start(mt[:], masks.ap().rearrange("i p n -> p i n"))
            if use_pad:
                pad_t = res.tile([P, NB], f32, tag="pad")
                nc.gpsimd.dma_start(pad_t[:], padm.ap())
            if use_vbias:
                bvr = res.tile([1, HD], f32, tag="bvr")
                nc.gpsimd.dma_start(bvr[:], bv.ap()[None, :])
                ones_row = res.tile([1, P], bf16, tag="or")
                nc.gpsimd.memset(ones_row[:], 1.0)
                bvb = res.tile([P, HD], f32, tag="bvb")

            # ---- Q-rot: q~[m', sq] = A^T @ qT, bf16 out ----
            with tc.tile_pool(name="qr", bufs=4, space="PSUM") as qrp:
                # HAM warmup: ~3.4us of throwaway matmuls while input DMAs
                # stream, so real matmuls start at the full 2.4 GHz clock.
                wps = qrp.tile([P, P], f32, tag="q", name="wps")
                for w in range(32):
                    nc.tensor.matmul(wps[:], wmt[:], wmt[:],
                                     start=(w == 0), stop=(w == 31))
                for sqc in range(SQL // 512):
                    for hb in range(MC):
                        ps = qrp.tile([P, 512], f32, tag="q",
                                      name=f"qp{sqc}_{hb}")
                        for mc in range(MC):
                            nc.tensor.matmul(
                                ps[:], ablk(hb, mc),
                                qx[:, sqc, mc, :],
                                start=(mc == 0), stop=(mc == MC - 1))
                        if hb % 2 == 0:
                            nc.vector.tensor_copy(
                                qh[:, hb, sqc * 512:(sqc + 1) * 512], ps[:])
                        else:
                            nc.scalar.copy(
                                qh[:, hb, sqc * 512:(sqc + 1) * 512], ps[:])
                if use_vbias:
                    # broadcast bv across partitions once: ones_row^T @ bvr
                    for hc in range(2):
                        bps = qrp.tile([P, 512], f32, tag="q", name=f"bb{hc}")
                        nc.tensor.matmul(bps[:], ones_row[:],
                                         bvr[:, hc * 512:(hc + 1) * 512],
                                         start=True, stop=True)
                        nc.vector.tensor_copy(
                            bvb[:, hc * 512:(hc + 1) * 512], bps[:])

            # ---- attention, pair pj = chunks (2pj, 2pj+1) = 512 queries ----
            # Shared keys kb < 8pj+4 run N=512 over both chunks' queries; the
            # odd chunk's extra band kb in [8pj+4, 8pj+8) runs N=256 into the
            # upper half of the same PSUM banks (no new accumulation group).
            with (
                tc.tile_pool(name="sc", bufs=2, space="PSUM") as scp,
                tc.tile_pool(name="ev", bufs=4, space="PSUM") as evp,
                tc.tile_pool(name="fin", bufs=2, space="PSUM") as finp,
            ):
                pending = []

                def e_slice(pj, esh, eex, kb, tb):
                    """e[kb] columns for query-block tb (0..3) of pair pj."""
                    if kb < 8 * pj + 4:
                        return esh[kb][:, tb * P:(tb + 1) * P]
                    return eex[kb - 8 * pj - 4][:, (tb - 2) * P:(tb - 1) * P]

                def make_finals(pj, esh, eex, evt):
                    """Deferred denominator + final-GEMM thunks for pair pj,
                    interleaved into the next pair's score loop."""
                    st = {}

                    def dn_chain(tb):
                        # chunk-local key range: even chunk (tb 0,1) stops at
                        # the shared band; odd chunk (tb 2,3) includes extras.
                        E = 8 * pj + 4 if tb < 2 else 8 * pj + 8
                        def f():
                            dps = scp.tile([P, 512], f32, tag="s",
                                           name=f"d{pj}_{tb}")
                            for kb in range(E):
                                nc.tensor.matmul(
                                    dps[:, 0:2], e_slice(pj, esh, eex, kb, tb),
                                    ones[:], start=(kb == 0),
                                    stop=(kb == E - 1))
                            dr = small.tile([P, 2], f32, tag="dr",
                                            name=f"dr{pj}_{tb}")
                            nc.vector.reciprocal(dr[:, 1:2], dps[:, 0:1])
                            st[tb] = dr[:, 1:2]
                            st[(tb, "o")] = outp.tile([P, HD], f32, tag="o",
                                                      name=f"o{pj}_{tb}")
                        return f

                    def fin_group(tb, hc):
                        def f():
                            fps = finp.tile([P, 512], f32, tag="f",
                                            name=f"f{pj}_{tb}_{hc}")
                            for mc in range(MC):
                                nc.tensor.matmul(
                                    fps[:], evt[:, mc, tb * P:(tb + 1) * P],
                                    wv_sb[:, mc, hc * 512:(hc + 1) * 512],
                                    start=(mc == 0), stop=(mc == MC - 1))
                            o = st[(tb, "o")]
                            rr = st[tb]
                            osl = o[:, hc * 512:(hc + 1) * 512]
                            if hc == 0:
                                nc.vector.tensor_scalar_mul(osl, fps[:], rr)
                            else:
                                nc.scalar.activation(osl, fps[:], Act.Copy,
                                                     scale=rr)
                            if use_vbias:
                                nc.vector.tensor_add(
                                    osl, osl, bvb[:, hc * 512:(hc + 1) * 512])
                            if hc == 1:
                                # query block tb of pair pj is local block
                                # lb: chunk 2pj+tb//2... blocks are
                                # (chunk j, t) -> 2j + t with j = 2pj + tb//2,
                                # t = tb % 2
                                lb = 4 * pj + 2 * (tb // 2) + tb % 2
                                nc.sync.dma_start(
                                    out.ap()[lb * P:(lb + 1) * P, :], o[:])
                        return f

                    dns = [dn_chain(tb) for tb in range(4)]
                    fins = [fin_group(tb, hc) for tb in range(4)
                            for hc in range(2)]
                    return dns, fins

                for pj in range(2):
                    ES = 8 * pj + 4     # shared kb count
                    sq0 = pj * 512
                    esh, eex = [], []
                    own_dns = None
                    evt = evs.tile([P, MC, 512], bf16, tag="evt",
                                   name=f"evt{pj}")
                    p0 = [evp.tile([P, 512], f32, tag="ev",
                                   name=f"ev{pj}_0_{i}") for i in range(2)]

                    def scores_kb(kb, ex_extra):
                        """One key block: matmuls + exp (+mask). Returns the
                        bf16 e tile ([P,512] shared or [P,256] extra)."""
                        if not ex_extra:
                            sps = scp.tile([P, 512], f32, tag="s",
                                           name=f"s{pj}_{kb}")
                            for mc in range(MC):
                                nc.tensor.matmul(
                                    sps[:], kblk(kb, mc),
                                    qh[:, mc, sq0:sq0 + 512],
                                    start=(mc == 0), stop=(mc == MC - 1))
                            ex = epool.tile([P, 512], bf16, tag="e", bufs=18,
                                            name=f"e{pj}_{kb}")
                            nc.scalar.activation(ex[:], sps[:], Act.Exp,
                                                 scale=1.0 / 32.0)
                            if kb >= 8 * pj:
                                nc.vector.tensor_mul(
                                    ex[:, 0:256], ex[:, 0:256],
                                    mt[:, kb - 8 * pj, :])
                        else:
                            sps = scp.tile([P, 512], f32, tag="s",
                                           name=f"sx{pj}_{kb}")
                            for mc in range(MC):
                                nc.tensor.matmul(
                                    sps[:, 0:256], kblk(kb, mc),
                                    qh[:, mc, sq0 + 256:sq0 + 512],
                                    start=(mc == 0), stop=(mc == MC - 1))
                            ex = epool.tile([P, 256], bf16, tag="ee", bufs=10,
                                            name=f"ex{pj}_{kb}")
                            nc.scalar.activation(ex[:], sps[:, 0:256], Act.Exp,
                                                 scale=1.0 / 32.0)
                            nc.vector.tensor_mul(ex[:], ex[:],
                                                 mt[:, kb - ES, :])
                        if use_pad:
                            nc.vector.tensor_scalar_mul(
                                ex[:], ex[:], pad_t[:, kb:kb + 1])
                        return ex

                    # score loop over all kb (shared then extra), with the
                    # evT m-chunk pass 0 riding one kb behind, and previous
                    # pair's finals filling remaining slots.
                    ET = ES + 4
                    for kb in range(ET):
                        ex = scores_kb(kb, kb >= ES)
                        (esh if kb < ES else eex).append(ex)
                        if kb > 0:
                            pk = kb - 1
                            for i in range(2):
                                if pk < ES:
                                    nc.tensor.matmul(
                                        p0[i][:], v_sb[:, pk, i * P:(i + 1) * P],
                                        esh[pk][:], start=(pk == 0),
                                        stop=False)
                                else:
                                    nc.tensor.matmul(
                                        p0[i][:, 256:512],
                                        v_sb[:, pk, i * P:(i + 1) * P],
                                        eex[pk - ES][:], start=False,
                                        stop=False)
                        if pending:
                            pending.pop(0)()
                    for i in range(2):
                        nc.tensor.matmul(
                            p0[i][:, 256:512], v_sb[:, ET - 1, i * P:(i + 1) * P],
                            eex[3][:], start=False, stop=True)
                        if i == 0:
                            nc.vector.tensor_copy(evt[:, i, :], p0[i][:])
                        else:
                            nc.scalar.copy(evt[:, i, :], p0[i][:])
                    for pp in range(1, 4):
                        pts = [evp.tile([P, 512], f32, tag="ev",
                                        name=f"ev{pj}_{pp}_{i}")
                               for i in range(2)]
                        for kb in range(ET):
                            for i in range(2):
                                mc = pp * 2 + i
                                if kb < ES:
                                    nc.tensor.matmul(
                                        pts[i][:], v_sb[:, kb, mc * P:(mc + 1) * P],
                                        esh[kb][:], start=(kb == 0),
                                        stop=False)
                                else:
                                    nc.tensor.matmul(
                                        pts[i][:, 256:512],
                                        v_sb[:, kb, mc * P:(mc + 1) * P],
                                        eex[kb - ES][:], start=False,
                                        stop=(kb == ET - 1))
                        for i in range(2):
                            mc = pp * 2 + i
                            if i == 0:
                                nc.vector.tensor_copy(evt[:, mc, :], pts[i][:])
                            else:
                                nc.scalar.copy(evt[:, mc, :], pts[i][:])
                        # last pair: its denominator chains slot between evT
                        # passes (they only read e tiles)
                        if pj == 1:
                            if own_dns is None:
                                own_dns, own_fins = make_finals(
                                    pj, esh, eex, evt)
                            if own_dns:
                                own_dns.pop(0)()
                    while pending:
                        pending.pop(0)()
                    if pj == 0:
                        dns, fins = make_finals(pj, esh, eex, evt)
                        pending = []
                        for tb in range(4):
                            pending.append(dns[tb])
                            pending.append(fins[2 * tb])
                            pending.append(fins[2 * tb + 1])
                    else:
                        while own_dns:
                            own_dns.pop(0)()
                        for f in own_fins:
                            f()

    nc.compile()
    return nc

def kernel(q, k, v, attention_mask, Wq_w, Wq_b, Wk_w, Wk_b, Wv_w, Wv_b):
    import ml_dtypes
    from concourse.bass_utils import run_bass_kernel_spmd

    q = np.asarray(q, dtype=np.float32)
    k = np.asarray(k, dtype=np.float32)
    v = np.asarray(v, dtype=np.float32)
    am = np.asarray(attention_mask)

    use_pad = not bool((am == 1).all())
    use_vbias = bool(np.any(np.asarray(Wv_b) != 0))

    use_qkbias = bool(np.any(np.asarray(Wq_b) != 0) or np.any(np.asarray(Wk_b) != 0))
    if use_qkbias:
        nc = _build_general(use_pad, use_vbias)
    else:
        nc = _build_folded(use_pad, use_vbias)

    bf16 = ml_dtypes.bfloat16
    wvt = np.ascontiguousarray(np.asarray(Wv_w, np.float32).T)
    if not use_qkbias:
        A = (np.asarray(Wq_w, np.float64).T @ np.asarray(Wk_w, np.float64))
        A = np.ascontiguousarray(A.astype(np.float32))
    bq = np.ascontiguousarray(np.asarray(Wq_b, np.float32))
    bk = np.ascontiguousarray(np.asarray(Wk_b, np.float32))
    bv = np.ascontiguousarray(np.asarray(Wv_b, np.float32))

    # causal masks for the 4 tail key-blocks of each chunk, per half c.
    # entry [i, a, col]: key (4j+i)*128+a vs query (4j+c+2t)*128+b, t=col//128.
    mask_c = []
    a = np.arange(P)[:, None]
    col = np.arange(256)[None, :]
    for c in range(2):
        t = col // P
        b_ = col % P
        m = np.stack([
            (128 * i + a <= 128 * (c + 2 * t) + b_) for i in range(4)
        ]).astype(np.float32)
        mask_c.append(m.astype(bf16))

    perms = []
    for c in range(2):
        perm = np.concatenate([
            np.arange(P) + (4 * j + c + 2 * t) * P
            for j in range(NCH) for t in range(2)
        ])
        perms.append(perm)

    in_maps = []
    if use_qkbias:
        wqt = np.ascontiguousarray(np.asarray(Wq_w, np.float32).T)
        wkt = np.ascontiguousarray(np.asarray(Wk_w, np.float32).T)
        kT = [np.ascontiguousarray(k[b].T) for b in range(B)]
        vT = [np.ascontiguousarray(v[b].T) for b in range(B)]
        for cid in range(N_CORES):
            b, c = cid // 2, cid % 2
            qT = np.ascontiguousarray(q[b].T[:, perms[c]])
            m = dict(qt=qT, kt=kT[b], vt=vT[b], wqt=wqt, wkt=wkt, wvt=wvt,
                     bq=bq, bk=bk, masks=mask_c[c])
            if use_pad:
                padv = am[b].astype(np.float32)
                m["padm"] = np.ascontiguousarray(padv.reshape(NB, P).T)
            if use_vbias:
                m["bv"] = bv
            in_maps.append(m)
    else:
        # pack so each device DMA is contiguous: x[q, p, mc, c] = xT[mc*128+p, q*W+c]
        def pack(xT, W):
            m2 = xT.shape[1] // W
            return np.ascontiguousarray(
                xT.reshape(8, 128, m2, W).transpose(2, 1, 0, 3))

        Ab = pack(A.astype(bf16), 256)
        wvtb = wvt.astype(bf16)
        kTb = [pack(np.ascontiguousarray(k[b].T).astype(bf16), 256)
               for b in range(B)]
        vnb = [np.ascontiguousarray(v[b]).astype(bf16) for b in range(B)]
        for cid in range(N_CORES):
            b, c = cid // 2, cid % 2
            qTb = pack(np.ascontiguousarray(q[b].T[:, perms[c]]).astype(bf16),
                       512)
            m = dict(qt=qTb, at=Ab, kt=kTb[b], vn=vnb[b], wvt=wvtb,
                     masks=mask_c[c])
            if use_pad:
                padv = am[b].astype(np.float32)
                m["padm"] = np.ascontiguousarray(padv.reshape(NB, P).T)
            if use_vbias:
                m["bv"] = bv
            in_maps.append(m)

    prof_dir = os.environ.get("ATTN_PROF_DIR")
    if prof_dir:
        try:
            from antenv.axon_hooks import get_axon_ntff_profile_hook
            hook = get_axon_ntff_profile_hook()
        except ImportError:
            hook = None
        if hook is None:
            from trn_agent_boot.trn_boot import _ntff_profile_via_ctypes
            hook = _ntff_profile_via_ctypes("/opt/axon/libaxon_pjrt.so")
        with hook(prof_dir, [0]):
            res = run_bass_kernel_spmd(nc, in_maps, list(range(N_CORES)))
    else:
        res = run_bass_kernel_spmd(nc, in_maps, list(range(N_CORES)))

    out = np.empty((B, S, HD), np.float32)
    for cid in range(N_CORES):
        b, c = cid // 2, cid % 2
        oc = res.results[cid]["out"]
        out[b, perms[c], :] = oc
    return out

